# revision 6
# baseline (speedup 1.0000x reference)
"""Trainium2 Bass kernel for hierarchical residual VQ (4 scales) + Phi conv.

Data-parallel over batch N=32: each of the 8 NeuronCores processes 4 batches.
Everything on-device is fp32 (matmuls use the float32r PE dtype: full fp32
precision at 1 cycle/row for moving-dim >= 256), so the codebook argmin matches
the fp32 reference.

Per core, per batch b (C=512 on partitions as 4 chunks of 128, T on free dim):
  residual <- x[b]
  for scale s in [1,2,4,8]:
    rdsum = blockwise sum of residual along T (no 1/s: folded into score bias)
    score[t,k] = rdsum_t . c_k - 0.5*s*||c_k||^2   (argmax == argmin distance)
      -> PE matmuls (f32r), K=1 trick adds the bias row; DVE max/max_index
    gather codebook rows by idx (indirect DMA), PE-transpose to (C,t)
    h' = -0.5 * linear_upsample(x_d)               (DVE strided phase ops)
    psum = conv1d(h', W) + I@h'                    (PE, f32r) == -0.5*(conv+up)
    residual[range] += psum + (-0.5*phi_b)         (one DVE scalar_tensor_tensor)
  f_hat = x - residual ; idx tensors DMA'd out per scale.
"""

import sys

sys.path.insert(0, "/opt/trn_rl_repo")

import numpy as np

P = 128
C = 512
CC = 4  # channel chunks of 128
T = 2048
K = 1024
NB = 4  # batches per core
NCORES = 8
SCALES = [1, 2, 4, 8]

_CACHE = {}


def _build_module():
    import concourse.bass as bass
    import concourse.mybir as mybir
    import concourse.tile as tile
    from concourse import bacc
    from concourse.bass import ts, ds, IndirectOffsetOnAxis
    from concourse.masks import make_identity

    f32 = mybir.dt.float32
    f32r = mybir.dt.float32r
    i32 = mybir.dt.int32
    u32 = mybir.dt.uint32

    nc = bacc.Bacc(
        "TRN2",
        target_bir_lowering=False,
        debug=False,
        enable_asserts=False,
        num_devices=NCORES,
    )

    # ---- DRAM I/O ----
    x_ap = nc.dram_tensor("x", (NB, C, T), f32, kind="ExternalInput").ap()
    cb_ap = nc.dram_tensor("cb", (K, C), f32, kind="ExternalInput").ap()
    cbt_ap = nc.dram_tensor("cbt", (P, CC, K), f32, kind="ExternalInput").ap()
    wt_ap = nc.dram_tensor("wt", (4, 3, CC, P, C), f32, kind="ExternalInput").ap()
    nb_ap = nc.dram_tensor("nb", (P, 4, CC), f32, kind="ExternalInput").ap()
    sbias_ap = nc.dram_tensor("sbias", (4, K), f32, kind="ExternalInput").ap()
    idx_aps = [
        nc.dram_tensor(f"idx{si}", (NB, T // s), i32, kind="ExternalOutput").ap()
        for si, s in enumerate(SCALES)
    ]
    fhat_ap = nc.dram_tensor("fhat", (NB, C, T), f32, kind="ExternalOutput").ap()

    def r32(ap):
        # plain fp32 matmul (2-pass, 4 cyc/row): exact, keeps argmin faithful.
        # float32r (1 cyc/row) measured ~TF32 precision -> flips argmins.
        return ap

    with tile.TileContext(nc) as tc:
        with (
            tc.tile_pool(name="consts", bufs=1) as consts,
            tc.tile_pool(name="wpool", bufs=1) as wpool,
            tc.tile_pool(name="resid", bufs=2) as residpool,
            tc.tile_pool(name="hbuf", bufs=1) as hpool,
            tc.tile_pool(name="xde", bufs=1) as xdepool,
            tc.tile_pool(name="rd", bufs=1) as rdpool,
            tc.tile_pool(name="sb", bufs=1) as sbpool,
            tc.tile_pool(name="xg", bufs=4) as xgpool,
            tc.tile_pool(name="small", bufs=4) as smallpool,
            tc.tile_pool(name="idxs", bufs=2) as idxpool,
            tc.tile_pool(name="pD", bufs=2, space="PSUM") as pDpool,
            tc.tile_pool(name="pT", bufs=2, space="PSUM") as pTpool,
            tc.tile_pool(name="pC", bufs=2, space="PSUM") as pCpool,
        ):
            # persistent constants
            cbt_sb = consts.tile([P, CC, K], f32)
            for cc in range(CC):
                nc.sync.dma_start(cbt_sb[:, cc, :], cbt_ap[:, cc, :])
            nb_sb = consts.tile([P, 4, CC], f32)
            nc.sync.dma_start(nb_sb[:, :, :], nb_ap[:, :, :])
            ones_sb = consts.tile([1, P], f32)
            nc.vector.memset(ones_sb[:], 1.0)
            ident = consts.tile([P, P], f32)
            make_identity(nc, ident[:])

            for b in range(NB):
                resid = residpool.tile([P, CC, T], f32, tag="resid")
                for cc in range(CC):
                    nc.sync.dma_start(
                        resid[:, cc, :], x_ap[b, ds(cc * P, P), :]
                    )

                for si, s in enumerate(SCALES):
                    Td = T // s
                    ntt = Td // P

                    # conv weights + score bias for this scale
                    wsb = wpool.tile([P, 3, CC, C], f32, tag="w")
                    for cc in range(CC):
                        nc.sync.dma_start(
                            wsb[:, :, cc, :],
                            wt_ap[si, :, cc].rearrange("k p co -> p k co"),
                        )
                    sbias_sb = sbpool.tile([1, K], f32, tag="sbias")
                    nc.sync.dma_start(sbias_sb[0:1, :], sbias_ap[si][None, :])

                    # ---- downsample (block sum along T) ----
                    if s == 1:
                        rdsrc = resid
                    else:
                        rd = rdpool.tile([P, CC, T // 2], f32, tag="rd")
                        nc.vector.tensor_reduce(
                            out=rd[:, :, :Td],
                            in_=resid[:].rearrange("p c (t s) -> p c t s", s=s),
                            axis=mybir.AxisListType.X,
                            op=mybir.AluOpType.add,
                        )
                        rdsrc = rd

                    # ---- distance + argmin + gather + transpose ----
                    hb = hpool.tile([P, CC, T + 2], f32, tag="h")
                    nc.vector.memset(hb[:, :, 0:1], 0.0)
                    nc.vector.memset(hb[:, :, T + 1 : T + 2], 0.0)
                    if s > 1:
                        xde = xdepool.tile([P, CC, T // 2 + 2], f32, tag="xde")
                    idx_sb = idxpool.tile([P, 16], i32, tag="idx")

                    for tt in range(ntt):
                        D = pDpool.tile([P, K], f32, tag="D")
                        for half in range(2):
                            Dh = D[:, ds(half * 512, 512)]
                            nc.tensor.matmul(
                                Dh,
                                r32(ones_sb[0:1, :]),
                                r32(sbias_sb[0:1, ds(half * 512, 512)]),
                                start=True,
                                stop=False,
                            )
                            for cc in range(CC):
                                nc.tensor.matmul(
                                    Dh,
                                    r32(rdsrc[:, cc, ts(tt, P)]),
                                    r32(cbt_sb[:, cc, ds(half * 512, 512)]),
                                    start=False,
                                    stop=(cc == CC - 1),
                                )
                        mx = smallpool.tile([P, 8], f32, tag="mx")
                        nc.vector.max(mx[:], D[:])
                        ix = smallpool.tile([P, 8], u32, tag="ix")
                        nc.vector.max_index(ix[:], mx[:], D[:])
                        nc.vector.tensor_copy(idx_sb[:, tt : tt + 1], ix[:, 0:1])

                        # gather codebook rows for these 128 positions
                        xg = xgpool.tile([P, C], f32, tag="xg")
                        nc.gpsimd.indirect_dma_start(
                            out=xg[:],
                            out_offset=None,
                            in_=cb_ap[:],
                            in_offset=IndirectOffsetOnAxis(ap=ix[:, 0:1], axis=0),
                        )
                        # transpose to (C, t) and place
                        for cc in range(CC):
                            pT = pTpool.tile([P, P], f32, tag="pT")
                            nc.tensor.transpose(pT[:], xg[:, ts(cc, P)], ident[:])
                            if s == 1:
                                nc.scalar.mul(
                                    hb[:, cc, 1 + tt * P : 1 + (tt + 1) * P],
                                    pT[:],
                                    -0.5,
                                )
                            else:
                                nc.scalar.copy(
                                    xde[:, cc, 1 + tt * P : 1 + (tt + 1) * P],
                                    pT[:],
                                )

                    # idx out
                    nc.sync.dma_start(
                        idx_aps[si][b].rearrange("(a p) -> p a", p=P),
                        idx_sb[:, :ntt],
                    )

                    # ---- upsample h' = -0.5 * up (s>1) ----
                    if s > 1:
                        nc.vector.tensor_copy(xde[:, :, 0:1], xde[:, :, 1:2])
                        nc.vector.tensor_copy(
                            xde[:, :, Td + 1 : Td + 2], xde[:, :, Td : Td + 1]
                        )
                        hview_all = hb[:, :, 1 : 1 + T].rearrange(
                            "p c (t s) -> p c t s", s=s
                        )
                        for ph in range(s):
                            phi = (ph + 0.5) / s - 0.5
                            if phi < 0:
                                delta, w1, w2 = 0, -phi, 1.0 + phi
                            else:
                                delta, w1, w2 = 1, 1.0 - phi, phi
                            hv = hview_all[:, :, :, ph]
                            x1 = xde[:, :, delta : delta + Td]
                            x2 = xde[:, :, delta + 1 : delta + 1 + Td]
                            nc.vector.tensor_scalar_mul(hv, x1, -0.5 * w1)
                            nc.vector.scalar_tensor_tensor(
                                out=hv,
                                in0=x2,
                                scalar=-0.5 * w2,
                                in1=hv,
                                op0=mybir.AluOpType.mult,
                                op1=mybir.AluOpType.add,
                            )

                    # ---- conv + residual update ----
                    for cc_out in range(CC):
                        for nt in range(T // 512):
                            pc = pCpool.tile([P, 512], f32, tag="pc")
                            nc.tensor.matmul(
                                pc[:],
                                r32(ident[:]),
                                r32(hb[:, cc_out, 1 + nt * 512 : 1 + nt * 512 + 512]),
                                start=True,
                                stop=False,
                            )
                            for k in range(3):
                                for ci in range(CC):
                                    nc.tensor.matmul(
                                        pc[:],
                                        r32(wsb[:, k, ci, ts(cc_out, P)]),
                                        r32(hb[:, ci, nt * 512 + k : nt * 512 + k + 512]),
                                        start=False,
                                        stop=(k == 2 and ci == CC - 1),
                                    )
                            nc.vector.scalar_tensor_tensor(
                                out=resid[:, cc_out, ts(nt, 512)],
                                in0=pc[:],
                                scalar=nb_sb[:, si, cc_out : cc_out + 1],
                                in1=resid[:, cc_out, ts(nt, 512)],
                                op0=mybir.AluOpType.add,
                                op1=mybir.AluOpType.add,
                            )

                # ---- f_hat = x - residual ----
                for cc in range(CC):
                    for nt in range(T // 512):
                        xt = xgpool.tile([P, C], f32, tag="xg")
                        nc.sync.dma_start(
                            xt[:], x_ap[b, ds(cc * P, P), ts(nt, 512)]
                        )
                        nc.vector.tensor_sub(
                            xt[:], xt[:], resid[:, cc, ts(nt, 512)]
                        )
                        nc.sync.dma_start(
                            fhat_ap[b, ds(cc * P, P), ts(nt, 512)], xt[:]
                        )

    nc.compile()
    return nc


def _host_prep(x, codebook, phi_w, phi_b):
    x = np.ascontiguousarray(np.asarray(x, dtype=np.float32))
    cb = np.ascontiguousarray(np.asarray(codebook, dtype=np.float32))
    pw = np.asarray(phi_w, dtype=np.float32)
    pb = np.asarray(phi_b, dtype=np.float32)

    cbt = np.ascontiguousarray(cb.T.reshape(CC, P, K).transpose(1, 0, 2))  # (P,CC,K)
    # wt[i,k,cc,p,co] = phi_w[i, co, cc*128+p, k]
    wt = np.ascontiguousarray(
        pw.transpose(0, 3, 2, 1).reshape(4, 3, CC, P, C)
    )
    nb = np.ascontiguousarray((-0.5 * pb).reshape(4, CC, P).transpose(2, 0, 1))  # (P,4,CC)
    cnorm = (cb.astype(np.float64) ** 2).sum(-1)
    sbias = np.stack(
        [(-0.5 * s * cnorm).astype(np.float32) for s in SCALES]
    )  # (4,K)
    return x, cb, cbt, wt, nb, sbias


def kernel(x, codebook, phi_w, phi_b):
    from concourse import bass_utils

    x, cb, cbt, wt, nb, sbias = _host_prep(x, codebook, phi_w, phi_b)

    if "nc" not in _CACHE:
        _CACHE["nc"] = _build_module()
    nc = _CACHE["nc"]

    shared = {"cb": cb, "cbt": cbt, "wt": wt, "nb": nb, "sbias": sbias}
    in_maps = [
        {"x": x[c * NB : (c + 1) * NB], **shared} for c in range(NCORES)
    ]
    res = bass_utils.run_bass_kernel_spmd(nc, in_maps, core_ids=list(range(NCORES)))
    outs = res.results
    idxs = [
        np.concatenate([outs[c][f"idx{si}"] for c in range(NCORES)], axis=0)
        for si in range(4)
    ]
    fhat = np.concatenate([outs[c]["fhat"] for c in range(NCORES)], axis=0)
    return (*idxs, fhat)
